# revision 1
# baseline (speedup 1.0000x reference)
"""AvgDistanceConv (GNN message passing) on 8 Trainium2 NeuronCores.

out[:, 0] = pos = h[:, 0]
out[:, 1] = segment_mean over incoming edges of |pos[src] - pos[dst]|

Strategy
--------
Shard by *destination range*: core c owns nodes [c*12500, (c+1)*12500) and
processes exactly the edges pointing into them, so each core produces its
output slice independently -- no collectives needed (better than edge
sharding + all-reduce: same gather volume, zero communication).

Host prep is index-only (cast/sort/bincount/pad); all float work runs on
device:
  * per core, build a degree-sorted padded ELL table of src indices
    (per-128-row-tile K = max in-degree in tile; pad slots hold the row's
    own node id so they contribute |pos[n]-pos[n]| = 0),
  * device gathers pos[src] via indirect DMA (128 offsets/call -- the only
    dynamic-offset granularity this DGE config supports), subtracts the
    per-partition scalar pos[dst] (loaded from a host-staged degree-rank
    replica of pos, keeping the bottleneck engine free of non-edge work),
    abs-sum-reduces each row, divides by max(count, 1), and emits
    [pos, mean] pairs.
"""
import sys
sys.path.insert(0, '/opt/trn_rl_repo')
import numpy as np
import concourse.bass as bass
import concourse.mybir as mybir
from concourse.bass_utils import run_bass_kernel_spmd
from concourse.tile import TileContext

P = 128
NC = 8
N_NODES = 100000


def _split_sync_waits(nc, max_waits=1):
    """This walrus build rejects more than one sync wait per instruction.
    Hoist extras into standalone same-engine EventSemaphore waits placed
    immediately before the owning instruction (same-engine program order
    preserves the synchronization semantics)."""
    for f in nc.m.functions:
        for blk in f.blocks:
            insts = list(blk.instructions)
            new = []
            dirty = False
            for inst in insts:
                si = inst.sync_info
                if si is not None and len(si.on_wait) > max_waits:
                    waits = list(si.on_wait)
                    for j, w in enumerate(waits[:-max_waits]):
                        wi = mybir.InstEventSemaphore(
                            name=f"{inst.name}_hw{j}", ins=[], outs=[])
                        wi.engine = inst.engine
                        wi.sync_info = mybir.SyncInfo(on_wait=[w], on_update=[])
                        new.append(wi)
                    inst.sync_info = mybir.SyncInfo(
                        on_wait=waits[-max_waits:], on_update=list(si.on_update))
                    dirty = True
                new.append(inst)
            if dirty:
                blk.instructions = new


def _host_prep(h, src, dst):
    N = N_NODES
    NPC = N // NC
    TILES = (NPC + P - 1) // P
    ROWS = TILES * P
    E = src.shape[0]

    pos = np.ascontiguousarray(h[:, 0], dtype=np.float32).reshape(N, 1)
    src32 = src.astype(np.int32)
    dst32 = dst.astype(np.int32)

    cnt = np.bincount(dst32, minlength=N)
    order = np.argsort(dst32, kind='stable')
    ssrc = src32[order]
    starts = np.zeros(N + 1, np.int64)
    starts[1:] = np.cumsum(cnt)

    deg_c = cnt.reshape(NC, NPC)
    rank = np.argsort(-deg_c, axis=1, kind='stable')
    node_ids = rank + (np.arange(NC)[:, None] * NPC)
    deg_sorted = np.take_along_axis(deg_c, rank, axis=1)

    pad = ROWS - NPC
    node_ids_p = np.concatenate(
        [node_ids, np.repeat(np.arange(NC)[:, None] * NPC, pad, axis=1)], axis=1)
    deg_p = np.concatenate([deg_sorted, np.zeros((NC, pad), np.int64)], axis=1)

    # per-tile slot width, shared across cores (SPMD: one program for all)
    K_t = np.maximum(deg_p.reshape(NC, TILES, P).max(axis=(0, 2)), 1).astype(int)

    Kmax = int(K_t.max())
    ar = np.arange(Kmax)
    slot_idx = starts[node_ids_p][:, :, None] + ar[None, None, :]
    valid = ar[None, None, :] < deg_p[:, :, None]
    ell = np.where(valid, ssrc[np.minimum(slot_idx, E - 1)],
                   node_ids_p[:, :, None]).astype(np.int32)

    flat_ell = np.concatenate(
        [ell[:, t * P:(t + 1) * P, :K_t[t]].reshape(NC, -1) for t in range(TILES)],
        axis=1)
    cntf = deg_p.astype(np.float32)

    # degree-rank-permuted replica of pos (O(N) staging, like the pos
    # replica itself): gives each tile its pos[dst] column as a plain
    # contiguous load instead of an indirect gather on the bottleneck engine
    posr = pos[node_ids_p, 0].astype(np.float32)

    in_maps = []
    for c in range(NC):
        in_maps.append({
            "pos": pos,
            "ell": flat_ell[c],
            "posr": posr[c].reshape(-1, 1),
            "cntf": cntf[c].reshape(-1, 1),
        })
    meta = dict(N=N, NPC=NPC, TILES=TILES, ROWS=ROWS,
                K_t=K_t, S_total=int(flat_ell.shape[1]), node_ids=node_ids)
    return in_maps, meta


def _build_program(meta):
    N, TILES, ROWS, K_t, S_total = (meta["N"], meta["TILES"], meta["ROWS"],
                                    meta["K_t"], meta["S_total"])
    nc = bass.Bass()
    pos = nc.declare_dram_parameter("pos", [N, 1], mybir.dt.float32, isOutput=False)
    ell = nc.declare_dram_parameter("ell", [S_total], mybir.dt.int32, isOutput=False)
    posr = nc.declare_dram_parameter("posr", [ROWS, 1], mybir.dt.float32,
                                     isOutput=False)
    cntf = nc.declare_dram_parameter("cntf", [ROWS, 1], mybir.dt.float32,
                                     isOutput=False)
    out = nc.declare_dram_parameter("out", [ROWS, 2], mybir.dt.float32, isOutput=True)

    with TileContext(nc) as tc:
        with (
            tc.tile_pool(name="idxp", bufs=4) as idxp,
            tc.tile_pool(name="gp", bufs=6) as gp,
            tc.tile_pool(name="smallp", bufs=12) as smallp,
        ):
            off = 0
            for t in range(TILES):
                K = int(K_t[t])
                r0 = t * P
                idx_t = idxp.tile([P, K], mybir.dt.int32, tag="idx")
                nc.sync.dma_start(
                    out=idx_t[:],
                    in_=ell[off:off + P * K].rearrange("(p k) -> p k", p=P))
                cnt_t = smallp.tile([P, 1], mybir.dt.float32, tag="cnt")
                nc.sync.dma_start(out=cnt_t[:], in_=cntf[r0:r0 + P])

                posd = smallp.tile([P, 1], mybir.dt.float32, tag="posd")
                nc.sync.dma_start(out=posd[:], in_=posr[r0:r0 + P])

                g_t = gp.tile([P, K], mybir.dt.float32, tag="g")
                for k in range(K):
                    nc.gpsimd.indirect_dma_start(
                        out=g_t[:, k:k + 1], out_offset=None, in_=pos[:],
                        in_offset=bass.IndirectOffsetOnAxis(
                            ap=idx_t[:, k:k + 1], axis=0))

                nc.vector.tensor_scalar(
                    out=g_t[:], in0=g_t[:], scalar1=posd[:], scalar2=None,
                    op0=mybir.AluOpType.subtract)
                s_t = smallp.tile([P, 1], mybir.dt.float32, tag="s")
                nc.vector.tensor_reduce(
                    out=s_t[:], in_=g_t[:], axis=mybir.AxisListType.X,
                    op=mybir.AluOpType.add, apply_absolute_value=True)

                nc.vector.tensor_scalar_max(out=cnt_t[:], in0=cnt_t[:], scalar1=1.0)
                nc.vector.reciprocal(out=cnt_t[:], in_=cnt_t[:])
                o_t = smallp.tile([P, 2], mybir.dt.float32, tag="o")
                nc.vector.tensor_copy(out=o_t[:, 0:1], in_=posd[:])
                nc.vector.tensor_tensor(
                    out=o_t[:, 1:2], in0=s_t[:], in1=cnt_t[:],
                    op=mybir.AluOpType.mult)
                nc.sync.dma_start(out=out[r0:r0 + P], in_=o_t[:])
                off += P * K

    _split_sync_waits(nc)
    return nc


def kernel(h, src, dst):
    h = np.asarray(h)
    src = np.asarray(src)
    dst = np.asarray(dst)
    in_maps, meta = _host_prep(h, src, dst)
    nc = _build_program(meta)
    res = run_bass_kernel_spmd(nc, in_maps, list(range(NC)))
    N, NPC, node_ids = meta["N"], meta["NPC"], meta["node_ids"]
    final = np.empty((N, 2), np.float32)
    for c in range(NC):
        final[node_ids[c]] = res.results[c]["out"][:NPC]
    return final



# revision 2
# speedup vs baseline: 1.0027x; 1.0027x over previous
"""AvgDistanceConv (GNN message passing) on 8 Trainium2 NeuronCores.

out[:, 0] = pos = h[:, 0]
out[:, 1] = segment_mean over incoming edges of |pos[src] - pos[dst]|

Strategy
--------
Shard by destination range: core c owns nodes [c*12500, (c+1)*12500) and
processes exactly the edges pointing into them (no collectives).

The per-edge gather of pos[src] runs as GPSIMD ap_gather ucode (SBUF->SBUF,
measured 27.2 ns/idx per Q7 core, 8 cores in parallel) instead of
per-element indirect DMA (994 ns SWDGE overhead per 128 elements -> 8.9 ms).

Layout: the core's 12500 dst nodes are placed degree-sorted into 98
iterations x 128 partitions; partition 16g+r belongs to GPSIMD core
(group) g. Edges are split into 8 passes by src chunk of 12500 so each
pass's pos chunk fits the ap_gather table (<=32768 elems, int16 idx).
Per (pass, iteration) each group gathers the unpadded concatenation of its
16 channels' edge-source lists; ap_gather replicates the stream across the
group's 16 channels, so channel r picks out its own segment with a
host-staged 0/1 bf16 mask (index-derived): per iteration the DVE computes
|(v - pos_dst) * mask| and abs-sum-reduces to one scalar per channel.
Sums accumulate in f32; a final reciprocal-multiply divides by in-degree.
Host work is index-only plus O(N) float permutations of pos.
"""
import sys
sys.path.insert(0, '/opt/trn_rl_repo')
import numpy as np
import ml_dtypes
import concourse.bass as bass
import concourse.bacc as bacc
import concourse.mybir as mybir
from concourse.bass_utils import run_bass_kernel_spmd
from concourse.tile import TileContext

P = 128
NC = 8
N_NODES = 100000
NPC = N_NODES // NC          # 12500 dst nodes per core
ITERS = (NPC + P - 1) // P   # 98 iterations (12544 slots, 44 dummies)
PASSES = 8
CPC = N_NODES // PASSES      # 12500-wide src chunks per pass
IDX_CAP = 4096               # max num_idxs per ap_gather instruction
BF = ml_dtypes.bfloat16


def _split_sync_waits(nc, max_waits=1):
    """This walrus build rejects more than one sync wait per instruction.
    Hoist extras into standalone same-engine EventSemaphore waits placed
    immediately before the owning instruction (same-engine program order
    preserves the synchronization semantics)."""
    for f in nc.m.functions:
        for blk in f.blocks:
            insts = list(blk.instructions)
            new = []
            dirty = False
            for inst in insts:
                si = inst.sync_info
                if si is not None and len(si.on_wait) > max_waits:
                    waits = list(si.on_wait)
                    for j, w in enumerate(waits[:-max_waits]):
                        wi = mybir.InstEventSemaphore(
                            name=f"{inst.name}_hw{j}", ins=[], outs=[])
                        wi.engine = inst.engine
                        wi.sync_info = mybir.SyncInfo(on_wait=[w], on_update=[])
                        new.append(wi)
                    inst.sync_info = mybir.SyncInfo(
                        on_wait=waits[-max_waits:], on_update=list(si.on_update))
                    dirty = True
                new.append(inst)
            if dirty:
                blk.instructions = new


def _host_prep(h, src, dst):
    N = N_NODES
    pos = np.ascontiguousarray(h[:, 0], dtype=np.float32)
    src32 = src.astype(np.int32)
    dst32 = dst.astype(np.int32)

    deg = np.bincount(dst32, minlength=N)

    deg_c = deg.reshape(NC, NPC)
    rank = np.argsort(-deg_c, axis=1, kind='stable')          # [NC, NPC]
    node_ids = rank + (np.arange(NC)[:, None] * NPC)
    pad = ITERS * P - NPC
    node_ids_p = np.concatenate(
        [node_ids, np.repeat(np.arange(NC)[:, None] * NPC, pad, axis=1)],
        axis=1)
    nodes_gic = node_ids_p.reshape(NC, ITERS, P)              # node(c, it, ch)

    flat_rank = np.empty(N, np.int64)
    for c in range(NC):
        flat_rank[node_ids[c]] = np.arange(NPC)
    it_of = (flat_rank // P).astype(np.int32)
    ch_of = (flat_rank % P).astype(np.int32)

    W = pos[nodes_gic].transpose(0, 2, 1).copy()              # [NC, 128, 98]
    cntf = deg[nodes_gic].transpose(0, 2, 1).astype(np.float32)

    # per-edge placement
    e_core = dst32 // NPC
    e_it = it_of[dst32]
    e_ch = ch_of[dst32]
    e_grp = e_ch // 16
    e_r = e_ch % 16
    e_pass = src32 // CPC
    e_sidx = (src32 - e_pass * CPC).astype(np.int16)

    # group-stream length per (core, pass, group, iter) then shared width
    key = (((e_core.astype(np.int64) * PASSES + e_pass) * 8 + e_grp)
           * ITERS + e_it)
    glen = np.bincount(key, minlength=NC * PASSES * 8 * ITERS)
    glen = glen.reshape(NC, PASSES, 8, ITERS)
    # shared L per (pass, iter): max over cores and groups, rounded to 16
    L_pi = glen.max(axis=(0, 2))                              # [PASSES, ITERS]
    L_pi = ((L_pi + 15) // 16 * 16).astype(np.int64)

    # chunking: pack iterations so sum(L) <= IDX_CAP
    chunks = []                                               # per pass: (it0, its list end, Ls)
    for p in range(PASSES):
        ch_list = []
        it0 = 0
        while it0 < ITERS:
            tot = 0
            it1 = it0
            while it1 < ITERS and tot + max(int(L_pi[p, it1]), 16) <= IDX_CAP:
                tot += max(int(L_pi[p, it1]), 16)
                it1 += 1
            ch_list.append((it0, it1))
            it0 = it1
        chunks.append(ch_list)
    L_pi = np.maximum(L_pi, 16)

    # column offsets per (pass, iter) into the concatenated stream
    colof = np.zeros((PASSES, ITERS), np.int64)
    off = 0
    for p in range(PASSES):
        for (it0, it1) in chunks[p]:
            for it in range(it0, it1):
                colof[p, it] = off
                off += int(L_pi[p, it])
    total_cols = off                                          # slots per group

    # edge slot position: order edges by (core, pass, grp, it, r) and number
    order = np.lexsort((e_r, e_it, e_grp, e_pass, e_core))
    ks = key[order]
    run_start = np.r_[True, ks[1:] != ks[:-1]]
    pos_in_grp = np.arange(len(order)) - np.maximum.accumulate(
        np.where(run_start, np.arange(len(order)), 0))
    # slot of each (sorted) edge within its (pass, it) stream window
    oc = e_core[order]
    op_ = e_pass[order]
    og = e_grp[order]
    oi = e_it[order]
    orr = e_r[order]
    slot = colof[op_, oi] + pos_in_grp                        # [E] global col

    # sidx [NC, 128, total_cols/16] int16, wrapped per group;
    # mask [NC, 128, total_cols] bf16
    sidx = np.zeros((NC, P, total_cols // 16), np.int16)
    mask = np.zeros((NC, P, total_cols), BF)
    # wrapped position: stream slot t -> (partition 16g + t%16, col t//16)
    sidx[oc, 16 * og + slot % 16, slot // 16] = e_sidx[order]
    mask[oc, 16 * og + orr, slot] = 1.0

    # pass tables [NC, PASSES, 128, CPC] f32 (pos chunk replicated; pad slots
    # are masked so table[0] garbage is harmless)
    tbl = np.empty((NC, PASSES, P, CPC), np.float32)
    for p in range(PASSES):
        tbl[:, p, :, :] = pos[p * CPC:(p + 1) * CPC][None, None, :]

    in_maps = []
    for c in range(NC):
        in_maps.append({
            "tbl": tbl[c].reshape(PASSES * P, CPC),
            "sidx": sidx[c],
            "mask": mask[c],
            "wtab": W[c],
            "cntf": cntf[c],
        })
    meta = dict(chunks=chunks, L_pi=L_pi, colof=colof,
                total_cols=int(total_cols), nodes_gic=nodes_gic)
    return in_maps, meta


def _build_program(meta):
    chunks, L_pi, total_cols = meta["chunks"], meta["L_pi"], meta["total_cols"]
    nc = bacc.Bacc()
    tbl = nc.declare_dram_parameter("tbl", [PASSES * P, CPC],
                                    mybir.dt.float32, isOutput=False)
    sidx = nc.declare_dram_parameter("sidx", [P, total_cols // 16],
                                     mybir.dt.int16, isOutput=False)
    mask = nc.declare_dram_parameter("mask", [P, total_cols],
                                     mybir.dt.bfloat16, isOutput=False)
    wtab = nc.declare_dram_parameter("wtab", [P, ITERS], mybir.dt.float32,
                                     isOutput=False)
    cntf = nc.declare_dram_parameter("cntf", [P, ITERS], mybir.dt.float32,
                                     isOutput=False)
    out = nc.declare_dram_parameter("out", [P, 2 * ITERS], mybir.dt.float32,
                                    isOutput=True)
    outv = out[:].rearrange("p (b a) -> p b a", b=2)

    with TileContext(nc) as tc:
        with (
            tc.tile_pool(name="persist", bufs=1) as pers,
            tc.tile_pool(name="tblp", bufs=2) as tblp,
            tc.tile_pool(name="idxp", bufs=3) as idxp,
            tc.tile_pool(name="maskp", bufs=3) as maskp,
            tc.tile_pool(name="vp", bufs=3) as vp,
            tc.tile_pool(name="tbp", bufs=2) as tbp,
            tc.tile_pool(name="sp", bufs=2) as sp,
        ):
            t_w = pers.tile([P, ITERS], mybir.dt.float32, tag="t_w")
            t_cnt = pers.tile([P, ITERS], mybir.dt.float32, tag="t_cnt")
            t_s = pers.tile([P, ITERS], mybir.dt.float32, tag="t_s")
            nc.sync.dma_start(out=t_w[:], in_=wtab[:])
            nc.sync.dma_start(out=t_cnt[:], in_=cntf[:])
            nc.vector.memset(t_s[:], 0.0)

            off = 0
            for p in range(PASSES):
                t_tbl = tblp.tile([P, CPC], mybir.dt.float32, tag="tbl")
                nc.sync.dma_start(out=t_tbl[:], in_=tbl[p * P:(p + 1) * P])
                s_cols = sp.tile([P, ITERS], mybir.dt.float32, tag="scols")
                nc.vector.memset(s_cols[:], 0.0)
                for (it0, it1) in chunks[p]:
                    Ls = [int(L_pi[p, it]) for it in range(it0, it1)]
                    cols = sum(Ls)
                    si = idxp.tile([P, cols // 16], mybir.dt.int16, tag="si")
                    nc.sync.dma_start(out=si[:],
                                      in_=sidx[:, off // 16:(off + cols) // 16])
                    mk = maskp.tile([P, cols], mybir.dt.bfloat16, tag="mk")
                    nc.sync.dma_start(out=mk[:], in_=mask[:, off:off + cols])
                    v = vp.tile([P, cols], mybir.dt.float32, tag="v")
                    nc.gpsimd.ap_gather(out_ap=v[:], in_ap=t_tbl[:],
                                        idxs_ap=si[:], channels=P,
                                        num_elems=CPC, d=1, num_idxs=cols)
                    tb = tbp.tile([P, cols], mybir.dt.bfloat16, tag="tb")
                    co = 0
                    for k, it in enumerate(range(it0, it1)):
                        L = Ls[k]
                        nc.vector.tensor_scalar(
                            out=tb[:, co:co + L], in0=v[:, co:co + L],
                            scalar1=t_w[:, it:it + 1], scalar2=None,
                            op0=mybir.AluOpType.subtract)
                        nc.vector.tensor_tensor(
                            out=tb[:, co:co + L], in0=tb[:, co:co + L],
                            in1=mk[:, co:co + L], op=mybir.AluOpType.mult)
                        nc.vector.tensor_reduce(
                            out=s_cols[:, it:it + 1], in_=tb[:, co:co + L],
                            axis=mybir.AxisListType.X, op=mybir.AluOpType.add,
                            apply_absolute_value=True)
                        co += L
                    off += cols
                nc.vector.tensor_tensor(out=t_s[:], in0=t_s[:], in1=s_cols[:],
                                        op=mybir.AluOpType.add)

            nc.vector.tensor_scalar_max(out=t_cnt[:], in0=t_cnt[:],
                                        scalar1=1.0)
            nc.vector.reciprocal(out=t_cnt[:], in_=t_cnt[:])
            nc.vector.tensor_tensor(out=t_s[:], in0=t_s[:], in1=t_cnt[:],
                                    op=mybir.AluOpType.mult)
            nc.sync.dma_start(out=outv[:, 0], in_=t_w[:])
            nc.sync.dma_start(out=outv[:, 1], in_=t_s[:])

    nc.compile()
    _split_sync_waits(nc)
    return nc


def kernel(h, src, dst):
    h = np.asarray(h)
    src = np.asarray(src)
    dst = np.asarray(dst)
    in_maps, meta = _host_prep(h, src, dst)
    nc = _build_program(meta)
    res = run_bass_kernel_spmd(nc, in_maps, list(range(NC)))
    nodes_gic = meta["nodes_gic"]
    final = np.empty((N_NODES, 2), np.float32)
    for c in range(NC):
        r = res.results[c]["out"].reshape(P, 2, ITERS)
        flat_nodes = nodes_gic[c].reshape(-1)
        vals = r.transpose(2, 0, 1).reshape(-1, 2)     # (it, ch) order
        final[flat_nodes[:NPC]] = vals[:NPC]
    return final


# revision 3
# speedup vs baseline: 1.0801x; 1.0772x over previous
"""AvgDistanceConv (GNN message passing) on 8 Trainium2 NeuronCores.

out[:, 0] = pos = h[:, 0]
out[:, 1] = segment_mean over incoming edges of |pos[src] - pos[dst]|

Strategy
--------
Shard by destination range: core c owns nodes [c*12500, (c+1)*12500) and
processes exactly the edges pointing into them (no collectives).

The per-edge gather of pos[src] runs as GPSIMD ap_gather ucode (SBUF->SBUF,
measured 27.2 ns/idx per Q7 core, 8 cores in parallel) instead of
per-element indirect DMA (994 ns SWDGE overhead per 128 elements -> 8.9 ms).

Layout: the core's 12500 dst nodes are placed degree-sorted into 98
iterations x 128 partitions; partition 16g+r belongs to GPSIMD core
(group) g. Edges are split into 8 passes by src chunk of 12500 so each
pass's pos chunk fits the ap_gather table (<=32768 elems, int16 idx).
Per (pass, iteration) each group gathers the unpadded concatenation of its
16 channels' edge-source lists; ap_gather replicates the stream across the
group's 16 channels, so channel r picks out its own segment with a
host-staged 0/1 bf16 mask (index-derived): per iteration the DVE computes
|(v - pos_dst) * mask| and abs-sum-reduces to one scalar per channel.
Sums accumulate in f32; a final reciprocal-multiply divides by in-degree.
Host work is index-only plus O(N) float permutations of pos.
"""
import sys
sys.path.insert(0, '/opt/trn_rl_repo')
import numpy as np
import ml_dtypes
import concourse.bass as bass
import concourse.bacc as bacc
import concourse.mybir as mybir
from concourse.bass_utils import run_bass_kernel_spmd
from concourse.tile import TileContext

P = 128
NC = 8
N_NODES = 100000
NPC = N_NODES // NC          # 12500 dst nodes per core
ITERS = (NPC + P - 1) // P   # 98 iterations (12544 slots, 44 dummies)
PASSES = 8
CPC = N_NODES // PASSES      # 12500-wide src chunks per pass
IDX_CAP = 4096               # max num_idxs per ap_gather instruction
BF = ml_dtypes.bfloat16


def _split_sync_waits(nc, max_waits=1):
    """This walrus build rejects more than one sync wait per instruction.
    Hoist extras into standalone same-engine EventSemaphore waits placed
    immediately before the owning instruction (same-engine program order
    preserves the synchronization semantics)."""
    for f in nc.m.functions:
        for blk in f.blocks:
            insts = list(blk.instructions)
            new = []
            dirty = False
            for inst in insts:
                si = inst.sync_info
                if si is not None and len(si.on_wait) > max_waits:
                    waits = list(si.on_wait)
                    for j, w in enumerate(waits[:-max_waits]):
                        wi = mybir.InstEventSemaphore(
                            name=f"{inst.name}_hw{j}", ins=[], outs=[])
                        wi.engine = inst.engine
                        wi.sync_info = mybir.SyncInfo(on_wait=[w], on_update=[])
                        new.append(wi)
                    inst.sync_info = mybir.SyncInfo(
                        on_wait=waits[-max_waits:], on_update=list(si.on_update))
                    dirty = True
                new.append(inst)
            if dirty:
                blk.instructions = new


def _host_prep(h, src, dst):
    N = N_NODES
    pos = np.ascontiguousarray(h[:, 0], dtype=np.float32)
    src32 = src.astype(np.int32)
    dst32 = dst.astype(np.int32)

    deg = np.bincount(dst32, minlength=N)

    deg_c = deg.reshape(NC, NPC)
    rank = np.argsort(-deg_c, axis=1, kind='stable')          # [NC, NPC]
    node_ids = rank + (np.arange(NC)[:, None] * NPC)
    pad = ITERS * P - NPC
    # pad with sentinel node id N (zero-degree dummy; posx/degx extended)
    nodes_rank = np.concatenate(
        [node_ids, np.full((NC, pad), N, np.int64)], axis=1
    ).reshape(NC, ITERS, P)

    posx = np.append(pos, np.float32(0.0))
    degx = np.append(deg, 0)

    # per-(node, pass) in-degree, for balancing groups within an iteration
    e_pass0 = (src32 // CPC).astype(np.int64)
    degp = np.bincount(dst32.astype(np.int64) * PASSES + e_pass0,
                       minlength=(N + 1) * PASSES).reshape(N + 1, PASSES)

    # greedy LPT: assign each iteration's 128 nodes to 8 groups x 16 slots,
    # minimizing the max per-(group, pass) load (the shared window width L)
    nodes_gic = np.empty((NC, ITERS, P), np.int64)
    loads = np.zeros((NC, ITERS, 8, PASSES), np.int64)
    sizes = np.zeros((NC, ITERS, 8), np.int64)
    nd = degp[nodes_rank]                                     # [NC, IT, 128, PASSES]
    ci, ii = np.ogrid[:NC, :ITERS]
    for j in range(P):
        node_j = nodes_rank[:, :, j]                          # [NC, IT]
        d_j = nd[:, :, j, :]                                  # [NC, IT, PASSES]
        cand = loads + d_j[:, :, None, :]
        score = cand.max(axis=3) + (sizes >= 16) * (1 << 40)
        g = score.argmin(axis=2)                              # [NC, IT]
        slot = sizes[ci, ii, g]
        nodes_gic[ci, ii, 16 * g + slot] = node_j
        loads[ci, ii, g] += d_j
        sizes[ci, ii, g] += 1

    # inverse: real node -> (iteration, channel)
    it_of = np.empty(N + 1, np.int32)
    ch_of = np.empty(N + 1, np.int32)
    for c in range(NC):
        flat = nodes_gic[c].reshape(-1)
        it_of[flat] = np.arange(ITERS * P) // P
        ch_of[flat] = np.arange(ITERS * P) % P

    W = posx[nodes_gic].transpose(0, 2, 1).copy()             # [NC, 128, 98]
    cntf = degx[nodes_gic].transpose(0, 2, 1).astype(np.float32)

    # per-edge placement
    e_core = dst32 // NPC
    e_it = it_of[dst32]
    e_ch = ch_of[dst32]
    e_grp = e_ch // 16
    e_r = e_ch % 16
    e_pass = src32 // CPC
    e_sidx = (src32 - e_pass * CPC).astype(np.int16)

    # group-stream length per (core, pass, group, iter) then shared width
    key = (((e_core.astype(np.int64) * PASSES + e_pass) * 8 + e_grp)
           * ITERS + e_it)
    glen = np.bincount(key, minlength=NC * PASSES * 8 * ITERS)
    glen = glen.reshape(NC, PASSES, 8, ITERS)
    # shared L per (pass, iter): max over cores and groups, rounded to 16
    L_pi = glen.max(axis=(0, 2))                              # [PASSES, ITERS]
    L_pi = ((L_pi + 15) // 16 * 16).astype(np.int64)

    # chunking: pack iterations so sum(L) <= IDX_CAP
    chunks = []                                               # per pass: (it0, its list end, Ls)
    for p in range(PASSES):
        ch_list = []
        it0 = 0
        while it0 < ITERS:
            tot = 0
            it1 = it0
            while it1 < ITERS and tot + max(int(L_pi[p, it1]), 16) <= IDX_CAP:
                tot += max(int(L_pi[p, it1]), 16)
                it1 += 1
            ch_list.append((it0, it1))
            it0 = it1
        chunks.append(ch_list)
    L_pi = np.maximum(L_pi, 16)

    # column offsets per (pass, iter) into the concatenated stream
    colof = np.zeros((PASSES, ITERS), np.int64)
    off = 0
    for p in range(PASSES):
        for (it0, it1) in chunks[p]:
            for it in range(it0, it1):
                colof[p, it] = off
                off += int(L_pi[p, it])
    total_cols = off                                          # slots per group

    # edge slot position: order edges by (core, pass, grp, it, r) and number
    order = np.lexsort((e_r, e_it, e_grp, e_pass, e_core))
    ks = key[order]
    run_start = np.r_[True, ks[1:] != ks[:-1]]
    pos_in_grp = np.arange(len(order)) - np.maximum.accumulate(
        np.where(run_start, np.arange(len(order)), 0))
    # slot of each (sorted) edge within its (pass, it) stream window
    oc = e_core[order]
    op_ = e_pass[order]
    og = e_grp[order]
    oi = e_it[order]
    orr = e_r[order]
    slot = colof[op_, oi] + pos_in_grp                        # [E] global col

    # sidx [NC, 128, total_cols/16] int16, wrapped per group;
    # mask [NC, 128, total_cols] bf16
    sidx = np.zeros((NC, P, total_cols // 16), np.int16)
    mask = np.zeros((NC, P, total_cols), BF)
    # wrapped position: stream slot t -> (partition 16g + t%16, col t//16)
    sidx[oc, 16 * og + slot % 16, slot // 16] = e_sidx[order]
    mask[oc, 16 * og + orr, slot] = 1.0

    # pass tables [NC, PASSES, 128, CPC] f32 (pos chunk replicated; pad slots
    # are masked so table[0] garbage is harmless)
    tbl = np.empty((NC, PASSES, P, CPC), np.float32)
    for p in range(PASSES):
        tbl[:, p, :, :] = pos[p * CPC:(p + 1) * CPC][None, None, :]

    in_maps = []
    for c in range(NC):
        in_maps.append({
            "tbl": tbl[c].reshape(PASSES * P, CPC),
            "sidx": sidx[c],
            "mask": mask[c],
            "wtab": W[c],
            "cntf": cntf[c],
        })
    meta = dict(chunks=chunks, L_pi=L_pi, colof=colof,
                total_cols=int(total_cols), nodes_gic=nodes_gic)
    return in_maps, meta


def _build_program(meta):
    chunks, L_pi, total_cols = meta["chunks"], meta["L_pi"], meta["total_cols"]
    nc = bacc.Bacc()
    tbl = nc.declare_dram_parameter("tbl", [PASSES * P, CPC],
                                    mybir.dt.float32, isOutput=False)
    sidx = nc.declare_dram_parameter("sidx", [P, total_cols // 16],
                                     mybir.dt.int16, isOutput=False)
    mask = nc.declare_dram_parameter("mask", [P, total_cols],
                                     mybir.dt.bfloat16, isOutput=False)
    wtab = nc.declare_dram_parameter("wtab", [P, ITERS], mybir.dt.float32,
                                     isOutput=False)
    cntf = nc.declare_dram_parameter("cntf", [P, ITERS], mybir.dt.float32,
                                     isOutput=False)
    out = nc.declare_dram_parameter("out", [P, 2 * ITERS], mybir.dt.float32,
                                    isOutput=True)
    outv = out[:].rearrange("p (b a) -> p b a", b=2)

    with TileContext(nc) as tc:
        with (
            tc.tile_pool(name="persist", bufs=1) as pers,
            tc.tile_pool(name="tblp", bufs=2) as tblp,
            tc.tile_pool(name="idxp", bufs=3) as idxp,
            tc.tile_pool(name="maskp", bufs=3) as maskp,
            tc.tile_pool(name="vp", bufs=3) as vp,
            tc.tile_pool(name="tbp", bufs=2) as tbp,
            tc.tile_pool(name="sp", bufs=2) as sp,
        ):
            t_w = pers.tile([P, ITERS], mybir.dt.float32, tag="t_w")
            t_cnt = pers.tile([P, ITERS], mybir.dt.float32, tag="t_cnt")
            t_s = pers.tile([P, ITERS], mybir.dt.float32, tag="t_s")
            nc.sync.dma_start(out=t_w[:], in_=wtab[:])
            nc.sync.dma_start(out=t_cnt[:], in_=cntf[:])
            nc.vector.memset(t_s[:], 0.0)

            off = 0
            for p in range(PASSES):
                t_tbl = tblp.tile([P, CPC], mybir.dt.float32, tag="tbl")
                nc.sync.dma_start(out=t_tbl[:], in_=tbl[p * P:(p + 1) * P])
                s_cols = sp.tile([P, ITERS], mybir.dt.float32, tag="scols")
                nc.vector.memset(s_cols[:], 0.0)
                for (it0, it1) in chunks[p]:
                    Ls = [int(L_pi[p, it]) for it in range(it0, it1)]
                    cols = sum(Ls)
                    si = idxp.tile([P, cols // 16], mybir.dt.int16, tag="si")
                    nc.sync.dma_start(out=si[:],
                                      in_=sidx[:, off // 16:(off + cols) // 16])
                    mk = maskp.tile([P, cols], mybir.dt.bfloat16, tag="mk")
                    nc.sync.dma_start(out=mk[:], in_=mask[:, off:off + cols])
                    v = vp.tile([P, cols], mybir.dt.float32, tag="v")
                    nc.gpsimd.ap_gather(out_ap=v[:], in_ap=t_tbl[:],
                                        idxs_ap=si[:], channels=P,
                                        num_elems=CPC, d=1, num_idxs=cols)
                    tb = tbp.tile([P, cols], mybir.dt.bfloat16, tag="tb")
                    co = 0
                    for k, it in enumerate(range(it0, it1)):
                        L = Ls[k]
                        nc.vector.tensor_scalar(
                            out=tb[:, co:co + L], in0=v[:, co:co + L],
                            scalar1=t_w[:, it:it + 1], scalar2=None,
                            op0=mybir.AluOpType.subtract)
                        nc.vector.tensor_tensor(
                            out=tb[:, co:co + L], in0=tb[:, co:co + L],
                            in1=mk[:, co:co + L], op=mybir.AluOpType.mult)
                        nc.vector.tensor_reduce(
                            out=s_cols[:, it:it + 1], in_=tb[:, co:co + L],
                            axis=mybir.AxisListType.X, op=mybir.AluOpType.add,
                            apply_absolute_value=True)
                        co += L
                    off += cols
                nc.vector.tensor_tensor(out=t_s[:], in0=t_s[:], in1=s_cols[:],
                                        op=mybir.AluOpType.add)

            nc.vector.tensor_scalar_max(out=t_cnt[:], in0=t_cnt[:],
                                        scalar1=1.0)
            nc.vector.reciprocal(out=t_cnt[:], in_=t_cnt[:])
            nc.vector.tensor_tensor(out=t_s[:], in0=t_s[:], in1=t_cnt[:],
                                    op=mybir.AluOpType.mult)
            nc.sync.dma_start(out=outv[:, 0], in_=t_w[:])
            nc.sync.dma_start(out=outv[:, 1], in_=t_s[:])

    nc.compile()
    _split_sync_waits(nc)
    return nc


def kernel(h, src, dst):
    h = np.asarray(h)
    src = np.asarray(src)
    dst = np.asarray(dst)
    in_maps, meta = _host_prep(h, src, dst)
    nc = _build_program(meta)
    res = run_bass_kernel_spmd(nc, in_maps, list(range(NC)))
    nodes_gic = meta["nodes_gic"]
    final = np.empty((N_NODES, 2), np.float32)
    for c in range(NC):
        r = res.results[c]["out"].reshape(P, 2, ITERS)
        flat_nodes = nodes_gic[c].reshape(-1)          # (it, ch) order
        vals = r.transpose(2, 0, 1).reshape(-1, 2)
        valid = flat_nodes < N_NODES                   # drop sentinel dummies
        final[flat_nodes[valid]] = vals[valid]
    return final


# revision 4
# speedup vs baseline: 1.1314x; 1.0475x over previous
"""AvgDistanceConv (GNN message passing) on 8 Trainium2 NeuronCores.

out[:, 0] = pos = h[:, 0]
out[:, 1] = segment_mean over incoming edges of |pos[src] - pos[dst]|

Strategy
--------
Shard by destination range: core c owns nodes [c*12500, (c+1)*12500) and
processes exactly the edges pointing into them (no collectives).

The per-edge gather of pos[src] runs as GPSIMD ap_gather ucode (SBUF->SBUF,
measured 27.2 ns/idx per Q7 core, 8 cores in parallel) instead of
per-element indirect DMA (994 ns SWDGE overhead per 128 elements -> 8.9 ms).

Layout: the core's 12500 dst nodes are placed degree-sorted into 98
iterations x 128 partitions; partition 16g+r belongs to GPSIMD core
(group) g. Edges are split into 8 passes by src chunk of 12500 so each
pass's pos chunk fits the ap_gather table (<=32768 elems, int16 idx).
Per (pass, iteration) each group gathers the unpadded concatenation of its
16 channels' edge-source lists; ap_gather replicates the stream across the
group's 16 channels, so channel r picks out its own segment with a
host-staged 0/1 bf16 mask (index-derived): per iteration the DVE computes
|(v - pos_dst) * mask| and abs-sum-reduces to one scalar per channel.
Sums accumulate in f32; a final reciprocal-multiply divides by in-degree.
Host work is index-only plus O(N) float permutations of pos.
"""
import sys
sys.path.insert(0, '/opt/trn_rl_repo')
import numpy as np
import ml_dtypes
import concourse.bass as bass
import concourse.bacc as bacc
import concourse.mybir as mybir
from concourse.bass_utils import run_bass_kernel_spmd
from concourse.tile import TileContext

P = 128
NC = 8
N_NODES = 100000
NPC = N_NODES // NC          # 12500 dst nodes per core
ITERS = (NPC + P - 1) // P   # 98 iterations (12544 slots, 44 dummies)
PASSES = 8
CPC = N_NODES // PASSES      # 12500-wide src chunks per pass
IDX_CAP = 4096               # max num_idxs per ap_gather instruction
BF = ml_dtypes.bfloat16


def _split_sync_waits(nc, max_waits=1):
    """This walrus build rejects more than one sync wait per instruction.
    Hoist extras into standalone same-engine EventSemaphore waits placed
    immediately before the owning instruction (same-engine program order
    preserves the synchronization semantics)."""
    for f in nc.m.functions:
        for blk in f.blocks:
            insts = list(blk.instructions)
            new = []
            dirty = False
            for inst in insts:
                si = inst.sync_info
                if si is not None and len(si.on_wait) > max_waits:
                    waits = list(si.on_wait)
                    for j, w in enumerate(waits[:-max_waits]):
                        wi = mybir.InstEventSemaphore(
                            name=f"{inst.name}_hw{j}", ins=[], outs=[])
                        wi.engine = inst.engine
                        wi.sync_info = mybir.SyncInfo(on_wait=[w], on_update=[])
                        new.append(wi)
                    inst.sync_info = mybir.SyncInfo(
                        on_wait=waits[-max_waits:], on_update=list(si.on_update))
                    dirty = True
                new.append(inst)
            if dirty:
                blk.instructions = new


def _host_prep(h, src, dst):
    N = N_NODES
    pos = np.ascontiguousarray(h[:, 0], dtype=np.float32)
    src32 = src.astype(np.int32)
    dst32 = dst.astype(np.int32)

    deg = np.bincount(dst32, minlength=N)

    deg_c = deg.reshape(NC, NPC)
    rank = np.argsort(-deg_c, axis=1, kind='stable')          # [NC, NPC]
    node_ids = rank + (np.arange(NC)[:, None] * NPC)
    pad = ITERS * P - NPC
    # pad with sentinel node id N (zero-degree dummy; posx/degx extended)
    nodes_rank = np.concatenate(
        [node_ids, np.full((NC, pad), N, np.int64)], axis=1
    ).reshape(NC, ITERS, P)

    posx = np.append(pos, np.float32(0.0))
    degx = np.append(deg, 0)

    # per-(node, pass) in-degree, for balancing groups within an iteration
    e_pass0 = (src32 // CPC).astype(np.int64)
    degp = np.bincount(dst32.astype(np.int64) * PASSES + e_pass0,
                       minlength=(N + 1) * PASSES).reshape(N + 1, PASSES)

    # greedy LPT: assign each iteration's 128 nodes to 8 groups x 16 slots,
    # minimizing the max per-(group, pass) load (the shared window width L)
    nodes_gic = np.empty((NC, ITERS, P), np.int64)
    loads = np.zeros((NC, ITERS, 8, PASSES), np.int64)
    sizes = np.zeros((NC, ITERS, 8), np.int64)
    nd = degp[nodes_rank]                                     # [NC, IT, 128, PASSES]
    ci, ii = np.ogrid[:NC, :ITERS]
    for j in range(P):
        node_j = nodes_rank[:, :, j]                          # [NC, IT]
        d_j = nd[:, :, j, :]                                  # [NC, IT, PASSES]
        cand = loads + d_j[:, :, None, :]
        score = cand.max(axis=3) + (sizes >= 16) * (1 << 40)
        g = score.argmin(axis=2)                              # [NC, IT]
        slot = sizes[ci, ii, g]
        nodes_gic[ci, ii, 16 * g + slot] = node_j
        loads[ci, ii, g] += d_j
        sizes[ci, ii, g] += 1

    # inverse: real node -> (iteration, channel)
    it_of = np.empty(N + 1, np.int32)
    ch_of = np.empty(N + 1, np.int32)
    for c in range(NC):
        flat = nodes_gic[c].reshape(-1)
        it_of[flat] = np.arange(ITERS * P) // P
        ch_of[flat] = np.arange(ITERS * P) % P

    W = posx[nodes_gic].transpose(0, 2, 1).copy()             # [NC, 128, 98]
    cntf = degx[nodes_gic].transpose(0, 2, 1).astype(np.float32)

    # per-edge placement
    e_core = dst32 // NPC
    e_it = it_of[dst32]
    e_ch = ch_of[dst32]
    e_grp = e_ch // 16
    e_r = e_ch % 16
    e_pass = src32 // CPC
    e_sidx = (src32 - e_pass * CPC).astype(np.int16)

    # group-stream length per (core, pass, group, iter) then shared width
    key = (((e_core.astype(np.int64) * PASSES + e_pass) * 8 + e_grp)
           * ITERS + e_it)
    glen = np.bincount(key, minlength=NC * PASSES * 8 * ITERS)
    glen = glen.reshape(NC, PASSES, 8, ITERS)
    # shared L per (pass, iter): max over cores and groups (no per-window
    # rounding -- masks/sidx are absolute-slot addressed; only each chunk's
    # total is padded to a multiple of 16 for the wrapped idx layout)
    L_pi = np.maximum(glen.max(axis=(0, 2)), 1).astype(np.int64)

    # chunking: pack iterations so the padded sum <= IDX_CAP
    chunks = []                                               # per pass: (it0, it1)
    for p in range(PASSES):
        ch_list = []
        it0 = 0
        while it0 < ITERS:
            tot = 0
            it1 = it0
            while it1 < ITERS and ((tot + int(L_pi[p, it1]) + 15) // 16 * 16
                                   <= IDX_CAP):
                tot += int(L_pi[p, it1])
                it1 += 1
            ch_list.append((it0, it1))
            it0 = it1
        chunks.append(ch_list)

    # column offsets per (pass, iter); chunks padded to x16
    colof = np.zeros((PASSES, ITERS), np.int64)
    chunk_cols = {}                                           # (p, it0) -> padded cols
    off = 0
    for p in range(PASSES):
        for (it0, it1) in chunks[p]:
            c0 = off
            for it in range(it0, it1):
                colof[p, it] = off
                off += int(L_pi[p, it])
            off = (off + 15) // 16 * 16
            chunk_cols[(p, it0)] = off - c0
    total_cols = off                                          # slots per group

    # edge slot position: order edges by (core, pass, grp, it, r) and number
    order = np.lexsort((e_r, e_it, e_grp, e_pass, e_core))
    ks = key[order]
    run_start = np.r_[True, ks[1:] != ks[:-1]]
    pos_in_grp = np.arange(len(order)) - np.maximum.accumulate(
        np.where(run_start, np.arange(len(order)), 0))
    # slot of each (sorted) edge within its (pass, it) stream window
    oc = e_core[order]
    op_ = e_pass[order]
    og = e_grp[order]
    oi = e_it[order]
    orr = e_r[order]
    slot = colof[op_, oi] + pos_in_grp                        # [E] global col

    # sidx [NC, 128, total_cols/16] int16, wrapped per group;
    # mask [NC, 128, total_cols] bf16
    sidx = np.zeros((NC, P, total_cols // 16), np.int16)
    mask = np.zeros((NC, P, total_cols), BF)
    # wrapped position: stream slot t -> (partition 16g + t%16, col t//16)
    sidx[oc, 16 * og + slot % 16, slot // 16] = e_sidx[order]
    mask[oc, 16 * og + orr, slot] = 1.0

    # pass tables [NC, PASSES, 128, CPC] f32 (pos chunk replicated; pad slots
    # are masked so table[0] garbage is harmless)
    tbl = np.empty((NC, PASSES, P, CPC), np.float32)
    for p in range(PASSES):
        tbl[:, p, :, :] = pos[p * CPC:(p + 1) * CPC][None, None, :]

    in_maps = []
    for c in range(NC):
        in_maps.append({
            "tbl": tbl[c].reshape(PASSES * P, CPC),
            "sidx": sidx[c],
            "mask": mask[c],
            "wtab": W[c],
            "cntf": cntf[c],
        })
    meta = dict(chunks=chunks, L_pi=L_pi, colof=colof, chunk_cols=chunk_cols,
                total_cols=int(total_cols), nodes_gic=nodes_gic)
    return in_maps, meta


def _build_program(meta):
    chunks, L_pi, total_cols = meta["chunks"], meta["L_pi"], meta["total_cols"]
    chunk_cols = meta["chunk_cols"]
    nc = bacc.Bacc()
    tbl = nc.declare_dram_parameter("tbl", [PASSES * P, CPC],
                                    mybir.dt.float32, isOutput=False)
    sidx = nc.declare_dram_parameter("sidx", [P, total_cols // 16],
                                     mybir.dt.int16, isOutput=False)
    mask = nc.declare_dram_parameter("mask", [P, total_cols],
                                     mybir.dt.bfloat16, isOutput=False)
    wtab = nc.declare_dram_parameter("wtab", [P, ITERS], mybir.dt.float32,
                                     isOutput=False)
    cntf = nc.declare_dram_parameter("cntf", [P, ITERS], mybir.dt.float32,
                                     isOutput=False)
    out = nc.declare_dram_parameter("out", [P, 2 * ITERS], mybir.dt.float32,
                                    isOutput=True)
    outv = out[:].rearrange("p (b a) -> p b a", b=2)

    with TileContext(nc) as tc:
        with (
            tc.tile_pool(name="persist", bufs=1) as pers,
            tc.tile_pool(name="tblp", bufs=2) as tblp,
            tc.tile_pool(name="idxp", bufs=3) as idxp,
            tc.tile_pool(name="maskp", bufs=3) as maskp,
            tc.tile_pool(name="vp", bufs=3) as vp,
            tc.tile_pool(name="tbp", bufs=2) as tbp,
            tc.tile_pool(name="sp", bufs=2) as sp,
        ):
            t_w = pers.tile([P, ITERS], mybir.dt.float32, tag="t_w")
            t_cnt = pers.tile([P, ITERS], mybir.dt.float32, tag="t_cnt")
            t_s = pers.tile([P, ITERS], mybir.dt.float32, tag="t_s")
            nc.sync.dma_start(out=t_w[:], in_=wtab[:])
            nc.sync.dma_start(out=t_cnt[:], in_=cntf[:])
            nc.vector.memset(t_s[:], 0.0)

            off = 0
            for p in range(PASSES):
                t_tbl = tblp.tile([P, CPC], mybir.dt.float32, tag="tbl")
                nc.sync.dma_start(out=t_tbl[:], in_=tbl[p * P:(p + 1) * P])
                s_cols = sp.tile([P, ITERS], mybir.dt.float32, tag="scols")
                nc.vector.memset(s_cols[:], 0.0)
                for (it0, it1) in chunks[p]:
                    Ls = [int(L_pi[p, it]) for it in range(it0, it1)]
                    cols = chunk_cols[(p, it0)]
                    si = idxp.tile([P, cols // 16], mybir.dt.int16, tag="si")
                    nc.sync.dma_start(out=si[:],
                                      in_=sidx[:, off // 16:(off + cols) // 16])
                    mk = maskp.tile([P, cols], mybir.dt.bfloat16, tag="mk")
                    nc.sync.dma_start(out=mk[:], in_=mask[:, off:off + cols])
                    v = vp.tile([P, cols], mybir.dt.float32, tag="v")
                    nc.gpsimd.ap_gather(out_ap=v[:], in_ap=t_tbl[:],
                                        idxs_ap=si[:], channels=P,
                                        num_elems=CPC, d=1, num_idxs=cols)
                    tb = tbp.tile([P, cols], mybir.dt.bfloat16, tag="tb")
                    co = 0
                    for k, it in enumerate(range(it0, it1)):
                        L = Ls[k]
                        nc.vector.tensor_scalar(
                            out=tb[:, co:co + L], in0=v[:, co:co + L],
                            scalar1=t_w[:, it:it + 1], scalar2=None,
                            op0=mybir.AluOpType.subtract)
                        nc.vector.tensor_tensor(
                            out=tb[:, co:co + L], in0=tb[:, co:co + L],
                            in1=mk[:, co:co + L], op=mybir.AluOpType.mult)
                        nc.vector.tensor_reduce(
                            out=s_cols[:, it:it + 1], in_=tb[:, co:co + L],
                            axis=mybir.AxisListType.X, op=mybir.AluOpType.add,
                            apply_absolute_value=True)
                        co += L
                    off += cols
                nc.vector.tensor_tensor(out=t_s[:], in0=t_s[:], in1=s_cols[:],
                                        op=mybir.AluOpType.add)

            nc.vector.tensor_scalar_max(out=t_cnt[:], in0=t_cnt[:],
                                        scalar1=1.0)
            nc.vector.reciprocal(out=t_cnt[:], in_=t_cnt[:])
            nc.vector.tensor_tensor(out=t_s[:], in0=t_s[:], in1=t_cnt[:],
                                    op=mybir.AluOpType.mult)
            nc.sync.dma_start(out=outv[:, 0], in_=t_w[:])
            nc.sync.dma_start(out=outv[:, 1], in_=t_s[:])

    nc.compile()
    _split_sync_waits(nc)
    return nc


def kernel(h, src, dst):
    h = np.asarray(h)
    src = np.asarray(src)
    dst = np.asarray(dst)
    in_maps, meta = _host_prep(h, src, dst)
    nc = _build_program(meta)
    res = run_bass_kernel_spmd(nc, in_maps, list(range(NC)))
    nodes_gic = meta["nodes_gic"]
    final = np.empty((N_NODES, 2), np.float32)
    for c in range(NC):
        r = res.results[c]["out"].reshape(P, 2, ITERS)
        flat_nodes = nodes_gic[c].reshape(-1)          # (it, ch) order
        vals = r.transpose(2, 0, 1).reshape(-1, 2)
        valid = flat_nodes < N_NODES                   # drop sentinel dummies
        final[flat_nodes[valid]] = vals[valid]
    return final


# revision 5
# speedup vs baseline: 1.1456x; 1.0126x over previous
"""AvgDistanceConv (GNN message passing) on 8 Trainium2 NeuronCores.

out[:, 0] = pos = h[:, 0]
out[:, 1] = segment_mean over incoming edges of |pos[src] - pos[dst]|

Strategy
--------
Shard by destination range: core c owns nodes [c*12500, (c+1)*12500) and
processes exactly the edges pointing into them (no collectives).

The per-edge gather of pos[src] runs as GPSIMD ap_gather ucode (SBUF->SBUF,
measured 27.2 ns/idx per Q7 core, 8 cores in parallel) instead of
per-element indirect DMA (994 ns SWDGE overhead per 128 elements -> 8.9 ms).

Layout: the core's 12500 dst nodes are placed degree-sorted into 98
iterations x 128 partitions; partition 16g+r belongs to GPSIMD core
(group) g. Edges are split into 8 passes by src chunk of 12500 so each
pass's pos chunk fits the ap_gather table (<=32768 elems, int16 idx).
Per (pass, iteration) each group gathers the unpadded concatenation of its
16 channels' edge-source lists; ap_gather replicates the stream across the
group's 16 channels, so channel r picks out its own segment with a
host-staged 0/1 bf16 mask (index-derived): per iteration the DVE computes
|(v - pos_dst) * mask| and abs-sum-reduces to one scalar per channel.
Sums accumulate in f32; a final reciprocal-multiply divides by in-degree.
Host work is index-only plus O(N) float permutations of pos.
"""
import sys
sys.path.insert(0, '/opt/trn_rl_repo')
import numpy as np
import ml_dtypes
import concourse.bass as bass
import concourse.bacc as bacc
import concourse.mybir as mybir
from concourse.bass_utils import run_bass_kernel_spmd
from concourse.tile import TileContext

P = 128
NC = 8
N_NODES = 100000
NPC = N_NODES // NC          # 12500 dst nodes per core
ITERS = (NPC + P - 1) // P   # 98 iterations (12544 slots, 44 dummies)
PASSES = 8
CPC = N_NODES // PASSES      # 12500-wide src chunks per pass
IDX_CAP = 4096               # max num_idxs per ap_gather instruction
BF = ml_dtypes.bfloat16


def _split_sync_waits(nc, max_waits=1):
    """This walrus build rejects more than one sync wait per instruction.
    Hoist extras into standalone same-engine EventSemaphore waits placed
    immediately before the owning instruction (same-engine program order
    preserves the synchronization semantics)."""
    for f in nc.m.functions:
        for blk in f.blocks:
            insts = list(blk.instructions)
            new = []
            dirty = False
            for inst in insts:
                si = inst.sync_info
                if si is not None and len(si.on_wait) > max_waits:
                    waits = list(si.on_wait)
                    for j, w in enumerate(waits[:-max_waits]):
                        wi = mybir.InstEventSemaphore(
                            name=f"{inst.name}_hw{j}", ins=[], outs=[])
                        wi.engine = inst.engine
                        wi.sync_info = mybir.SyncInfo(on_wait=[w], on_update=[])
                        new.append(wi)
                    inst.sync_info = mybir.SyncInfo(
                        on_wait=waits[-max_waits:], on_update=list(si.on_update))
                    dirty = True
                new.append(inst)
            if dirty:
                blk.instructions = new


def _host_prep(h, src, dst):
    N = N_NODES
    pos = np.ascontiguousarray(h[:, 0], dtype=np.float32)
    src32 = src.astype(np.int32)
    dst32 = dst.astype(np.int32)

    deg = np.bincount(dst32, minlength=N)

    deg_c = deg.reshape(NC, NPC)
    rank = np.argsort(-deg_c, axis=1, kind='stable')          # [NC, NPC]
    node_ids = rank + (np.arange(NC)[:, None] * NPC)
    pad = ITERS * P - NPC
    # pad with sentinel node id N (zero-degree dummy; posx/degx extended)
    nodes_rank = np.concatenate(
        [node_ids, np.full((NC, pad), N, np.int64)], axis=1
    ).reshape(NC, ITERS, P)

    posx = np.append(pos, np.float32(0.0))
    degx = np.append(deg, 0)

    # per-(node, pass) in-degree, for balancing groups within an iteration
    e_pass0 = (src32 // CPC).astype(np.int64)
    degp = np.bincount(dst32.astype(np.int64) * PASSES + e_pass0,
                       minlength=(N + 1) * PASSES).reshape(N + 1, PASSES)

    # greedy LPT: assign each iteration's 128 nodes to 8 groups x 16 slots,
    # minimizing the max per-(group, pass) load (the shared window width L)
    nodes_gic = np.empty((NC, ITERS, P), np.int64)
    loads = np.zeros((NC, ITERS, 8, PASSES), np.int64)
    sizes = np.zeros((NC, ITERS, 8), np.int64)
    nd = degp[nodes_rank]                                     # [NC, IT, 128, PASSES]
    # LPT quality: place spiky nodes (large max per-pass degree) first
    ordj = np.argsort(-nd.max(axis=3), axis=2, kind='stable') # [NC, IT, 128]
    ci, ii = np.ogrid[:NC, :ITERS]
    for j in range(P):
        jj = ordj[:, :, j]
        node_j = nodes_rank[ci, ii, jj]                       # [NC, IT]
        d_j = nd[ci, ii, jj, :]                               # [NC, IT, PASSES]
        cand = loads + d_j[:, :, None, :]
        # minimize resulting max load; tie-break on total load
        score = (cand.max(axis=3) * 4096 + cand.sum(axis=3)
                 + (sizes >= 16) * (1 << 50))
        g = score.argmin(axis=2)                              # [NC, IT]
        slot = sizes[ci, ii, g]
        nodes_gic[ci, ii, 16 * g + slot] = node_j
        loads[ci, ii, g] += d_j
        sizes[ci, ii, g] += 1

    # inverse: real node -> (iteration, channel)
    it_of = np.empty(N + 1, np.int32)
    ch_of = np.empty(N + 1, np.int32)
    for c in range(NC):
        flat = nodes_gic[c].reshape(-1)
        it_of[flat] = np.arange(ITERS * P) // P
        ch_of[flat] = np.arange(ITERS * P) % P

    W = posx[nodes_gic].transpose(0, 2, 1).copy()             # [NC, 128, 98]
    cntf = degx[nodes_gic].transpose(0, 2, 1).astype(np.float32)

    # per-edge placement
    e_core = dst32 // NPC
    e_it = it_of[dst32]
    e_ch = ch_of[dst32]
    e_grp = e_ch // 16
    e_r = e_ch % 16
    e_pass = src32 // CPC
    e_sidx = (src32 - e_pass * CPC).astype(np.int16)

    # group-stream length per (core, pass, group, iter) then shared width
    key = (((e_core.astype(np.int64) * PASSES + e_pass) * 8 + e_grp)
           * ITERS + e_it)
    glen = np.bincount(key, minlength=NC * PASSES * 8 * ITERS)
    glen = glen.reshape(NC, PASSES, 8, ITERS)
    # shared L per (pass, iter): max over cores and groups (no per-window
    # rounding -- masks/sidx are absolute-slot addressed; only each chunk's
    # total is padded to a multiple of 16 for the wrapped idx layout)
    L_pi = np.maximum(glen.max(axis=(0, 2)), 1).astype(np.int64)

    # chunking: pack iterations so the padded sum <= IDX_CAP
    chunks = []                                               # per pass: (it0, it1)
    for p in range(PASSES):
        ch_list = []
        it0 = 0
        while it0 < ITERS:
            tot = 0
            it1 = it0
            while it1 < ITERS and ((tot + int(L_pi[p, it1]) + 15) // 16 * 16
                                   <= IDX_CAP):
                tot += int(L_pi[p, it1])
                it1 += 1
            ch_list.append((it0, it1))
            it0 = it1
        chunks.append(ch_list)

    # column offsets per (pass, iter); chunks padded to x16
    colof = np.zeros((PASSES, ITERS), np.int64)
    chunk_cols = {}                                           # (p, it0) -> padded cols
    off = 0
    for p in range(PASSES):
        for (it0, it1) in chunks[p]:
            c0 = off
            for it in range(it0, it1):
                colof[p, it] = off
                off += int(L_pi[p, it])
            off = (off + 15) // 16 * 16
            chunk_cols[(p, it0)] = off - c0
    total_cols = off                                          # slots per group

    # edge slot position: order edges by (core, pass, grp, it, r) and number
    order = np.lexsort((e_r, e_it, e_grp, e_pass, e_core))
    ks = key[order]
    run_start = np.r_[True, ks[1:] != ks[:-1]]
    pos_in_grp = np.arange(len(order)) - np.maximum.accumulate(
        np.where(run_start, np.arange(len(order)), 0))
    # slot of each (sorted) edge within its (pass, it) stream window
    oc = e_core[order]
    op_ = e_pass[order]
    og = e_grp[order]
    oi = e_it[order]
    orr = e_r[order]
    slot = colof[op_, oi] + pos_in_grp                        # [E] global col

    # sidx [NC, 128, total_cols/16] int16, wrapped per group;
    # mask [NC, 128, total_cols] bf16
    sidx = np.zeros((NC, P, total_cols // 16), np.int16)
    mask = np.zeros((NC, P, total_cols), BF)
    # wrapped position: stream slot t -> (partition 16g + t%16, col t//16)
    sidx[oc, 16 * og + slot % 16, slot // 16] = e_sidx[order]
    mask[oc, 16 * og + orr, slot] = 1.0

    # pass tables [NC, PASSES, 128, CPC] f32 (pos chunk replicated; pad slots
    # are masked so table[0] garbage is harmless)
    tbl = np.empty((NC, PASSES, P, CPC), np.float32)
    for p in range(PASSES):
        tbl[:, p, :, :] = pos[p * CPC:(p + 1) * CPC][None, None, :]

    in_maps = []
    for c in range(NC):
        in_maps.append({
            "tbl": tbl[c].reshape(PASSES * P, CPC),
            "sidx": sidx[c],
            "mask": mask[c],
            "wtab": W[c],
            "cntf": cntf[c],
        })
    meta = dict(chunks=chunks, L_pi=L_pi, colof=colof, chunk_cols=chunk_cols,
                total_cols=int(total_cols), nodes_gic=nodes_gic)
    return in_maps, meta


def _build_program(meta):
    chunks, L_pi, total_cols = meta["chunks"], meta["L_pi"], meta["total_cols"]
    chunk_cols = meta["chunk_cols"]
    nc = bacc.Bacc()
    tbl = nc.declare_dram_parameter("tbl", [PASSES * P, CPC],
                                    mybir.dt.float32, isOutput=False)
    sidx = nc.declare_dram_parameter("sidx", [P, total_cols // 16],
                                     mybir.dt.int16, isOutput=False)
    mask = nc.declare_dram_parameter("mask", [P, total_cols],
                                     mybir.dt.bfloat16, isOutput=False)
    wtab = nc.declare_dram_parameter("wtab", [P, ITERS], mybir.dt.float32,
                                     isOutput=False)
    cntf = nc.declare_dram_parameter("cntf", [P, ITERS], mybir.dt.float32,
                                     isOutput=False)
    out = nc.declare_dram_parameter("out", [P, 2 * ITERS], mybir.dt.float32,
                                    isOutput=True)
    outv = out[:].rearrange("p (b a) -> p b a", b=2)

    with TileContext(nc) as tc:
        with (
            tc.tile_pool(name="persist", bufs=1) as pers,
            tc.tile_pool(name="tblp", bufs=2) as tblp,
            tc.tile_pool(name="idxp", bufs=4) as idxp,
            tc.tile_pool(name="maskp", bufs=3) as maskp,
            tc.tile_pool(name="vp", bufs=3) as vp,
            tc.tile_pool(name="tbp", bufs=2) as tbp,
            tc.tile_pool(name="sp", bufs=2) as sp,
        ):
            t_w = pers.tile([P, ITERS], mybir.dt.float32, tag="t_w")
            t_cnt = pers.tile([P, ITERS], mybir.dt.float32, tag="t_cnt")
            t_s = pers.tile([P, ITERS], mybir.dt.float32, tag="t_s")
            # pass-0 table first: it gates the first gather
            t_tbl0 = tblp.tile([P, CPC], mybir.dt.float32, tag="tbl")
            nc.sync.dma_start(out=t_tbl0[:], in_=tbl[0:P])
            nc.sync.dma_start(out=t_w[:], in_=wtab[:])
            nc.sync.dma_start(out=t_cnt[:], in_=cntf[:])
            nc.vector.memset(t_s[:], 0.0)

            off = 0
            for p in range(PASSES):
                if p == 0:
                    t_tbl = t_tbl0
                else:
                    t_tbl = tblp.tile([P, CPC], mybir.dt.float32, tag="tbl")
                    nc.sync.dma_start(out=t_tbl[:],
                                      in_=tbl[p * P:(p + 1) * P])
                s_cols = sp.tile([P, ITERS], mybir.dt.float32, tag="scols")
                nc.vector.memset(s_cols[:], 0.0)
                for (it0, it1) in chunks[p]:
                    Ls = [int(L_pi[p, it]) for it in range(it0, it1)]
                    cols = chunk_cols[(p, it0)]
                    si = idxp.tile([P, cols // 16], mybir.dt.int16, tag="si")
                    nc.sync.dma_start(out=si[:],
                                      in_=sidx[:, off // 16:(off + cols) // 16])
                    mk = maskp.tile([P, cols], mybir.dt.bfloat16, tag="mk")
                    nc.sync.dma_start(out=mk[:], in_=mask[:, off:off + cols])
                    v = vp.tile([P, cols], mybir.dt.float32, tag="v")
                    nc.gpsimd.ap_gather(out_ap=v[:], in_ap=t_tbl[:],
                                        idxs_ap=si[:], channels=P,
                                        num_elems=CPC, d=1, num_idxs=cols)
                    tb = tbp.tile([P, cols], mybir.dt.bfloat16, tag="tb")
                    co = 0
                    for k, it in enumerate(range(it0, it1)):
                        L = Ls[k]
                        nc.vector.tensor_scalar(
                            out=tb[:, co:co + L], in0=v[:, co:co + L],
                            scalar1=t_w[:, it:it + 1], scalar2=None,
                            op0=mybir.AluOpType.subtract)
                        nc.vector.tensor_tensor(
                            out=tb[:, co:co + L], in0=tb[:, co:co + L],
                            in1=mk[:, co:co + L], op=mybir.AluOpType.mult)
                        nc.vector.tensor_reduce(
                            out=s_cols[:, it:it + 1], in_=tb[:, co:co + L],
                            axis=mybir.AxisListType.X, op=mybir.AluOpType.add,
                            apply_absolute_value=True)
                        co += L
                    off += cols
                nc.vector.tensor_tensor(out=t_s[:], in0=t_s[:], in1=s_cols[:],
                                        op=mybir.AluOpType.add)

            nc.vector.tensor_scalar_max(out=t_cnt[:], in0=t_cnt[:],
                                        scalar1=1.0)
            nc.vector.reciprocal(out=t_cnt[:], in_=t_cnt[:])
            nc.vector.tensor_tensor(out=t_s[:], in0=t_s[:], in1=t_cnt[:],
                                    op=mybir.AluOpType.mult)
            nc.sync.dma_start(out=outv[:, 0], in_=t_w[:])
            nc.sync.dma_start(out=outv[:, 1], in_=t_s[:])

    nc.compile()
    _split_sync_waits(nc)
    return nc


def kernel(h, src, dst):
    h = np.asarray(h)
    src = np.asarray(src)
    dst = np.asarray(dst)
    in_maps, meta = _host_prep(h, src, dst)
    nc = _build_program(meta)
    res = run_bass_kernel_spmd(nc, in_maps, list(range(NC)))
    nodes_gic = meta["nodes_gic"]
    final = np.empty((N_NODES, 2), np.float32)
    for c in range(NC):
        r = res.results[c]["out"].reshape(P, 2, ITERS)
        flat_nodes = nodes_gic[c].reshape(-1)          # (it, ch) order
        vals = r.transpose(2, 0, 1).reshape(-1, 2)
        valid = flat_nodes < N_NODES                   # drop sentinel dummies
        final[flat_nodes[valid]] = vals[valid]
    return final


# revision 6
# speedup vs baseline: 1.1569x; 1.0098x over previous
"""AvgDistanceConv (GNN message passing) on 8 Trainium2 NeuronCores.

out[:, 0] = pos = h[:, 0]
out[:, 1] = segment_mean over incoming edges of |pos[src] - pos[dst]|

Strategy
--------
Shard by destination range: core c owns nodes [c*12500, (c+1)*12500) and
processes exactly the edges pointing into them (no collectives).

The per-edge gather of pos[src] runs as GPSIMD ap_gather ucode (SBUF->SBUF,
measured 27.2 ns/idx per Q7 core, 8 cores in parallel) instead of
per-element indirect DMA (994 ns SWDGE overhead per 128 elements -> 8.9 ms).

Layout: the core's 12500 dst nodes are placed degree-sorted into 98
iterations x 128 partitions; partition 16g+r belongs to GPSIMD core
(group) g. Edges are split into 8 passes by src chunk of 12500 so each
pass's pos chunk fits the ap_gather table (<=32768 elems, int16 idx).
Per (pass, iteration) each group gathers the unpadded concatenation of its
16 channels' edge-source lists; ap_gather replicates the stream across the
group's 16 channels, so channel r picks out its own segment with a
host-staged 0/1 bf16 mask (index-derived): per iteration the DVE computes
|(v - pos_dst) * mask| and abs-sum-reduces to one scalar per channel.
Sums accumulate in f32; a final reciprocal-multiply divides by in-degree.
Host work is index-only plus O(N) float permutations of pos.
"""
import sys
sys.path.insert(0, '/opt/trn_rl_repo')
import numpy as np
import ml_dtypes
import concourse.bass as bass
import concourse.bacc as bacc
import concourse.mybir as mybir
from concourse.bass_utils import run_bass_kernel_spmd
from concourse.tile import TileContext

P = 128
NC = 8
N_NODES = 100000
NPC = N_NODES // NC          # 12500 dst nodes per core
ITERS = (NPC + P - 1) // P   # 98 iterations (12544 slots, 44 dummies)
PASSES = 8
CPC = N_NODES // PASSES      # 12500-wide src chunks per pass
IDX_CAP = 4096               # max num_idxs per ap_gather instruction
BF = ml_dtypes.bfloat16


def _split_sync_waits(nc, max_waits=1):
    """This walrus build rejects more than one sync wait per instruction.
    Hoist extras into standalone same-engine EventSemaphore waits placed
    immediately before the owning instruction (same-engine program order
    preserves the synchronization semantics)."""
    for f in nc.m.functions:
        for blk in f.blocks:
            insts = list(blk.instructions)
            new = []
            dirty = False
            for inst in insts:
                si = inst.sync_info
                if si is not None and len(si.on_wait) > max_waits:
                    waits = list(si.on_wait)
                    for j, w in enumerate(waits[:-max_waits]):
                        wi = mybir.InstEventSemaphore(
                            name=f"{inst.name}_hw{j}", ins=[], outs=[])
                        wi.engine = inst.engine
                        wi.sync_info = mybir.SyncInfo(on_wait=[w], on_update=[])
                        new.append(wi)
                    inst.sync_info = mybir.SyncInfo(
                        on_wait=waits[-max_waits:], on_update=list(si.on_update))
                    dirty = True
                new.append(inst)
            if dirty:
                blk.instructions = new


def _host_prep(h, src, dst):
    N = N_NODES
    pos = np.ascontiguousarray(h[:, 0], dtype=np.float32)
    src32 = src.astype(np.int32)
    dst32 = dst.astype(np.int32)

    deg = np.bincount(dst32, minlength=N)

    deg_c = deg.reshape(NC, NPC)
    rank = np.argsort(-deg_c, axis=1, kind='stable')          # [NC, NPC]
    node_ids = rank + (np.arange(NC)[:, None] * NPC)
    pad = ITERS * P - NPC
    # pad with sentinel node id N (zero-degree dummy; posx/degx extended)
    nodes_rank = np.concatenate(
        [node_ids, np.full((NC, pad), N, np.int64)], axis=1
    ).reshape(NC, ITERS, P)

    posx = np.append(pos, np.float32(0.0))
    degx = np.append(deg, 0)

    # per-(node, pass) in-degree, for balancing groups within an iteration
    e_pass0 = (src32 // CPC).astype(np.int64)
    degp = np.bincount(dst32.astype(np.int64) * PASSES + e_pass0,
                       minlength=(N + 1) * PASSES).reshape(N + 1, PASSES)

    # greedy LPT: assign each iteration's 128 nodes to 8 groups x 16 slots,
    # minimizing the max per-(group, pass) load (the shared window width L)
    nd = degp[nodes_rank]                                     # [NC, IT, 128, PASSES]
    ci, ii = np.ogrid[:NC, :ITERS]

    def greedy(ordj):
        """One LPT pass; returns (group choice per ordered node, score)."""
        loads = np.zeros((NC, ITERS, 8, PASSES), np.int64)
        sizes = np.zeros((NC, ITERS, 8), np.int64)
        gsel = np.empty((NC, ITERS, P), np.int8)
        for j in range(P):
            jj = ordj[:, :, j]
            d_j = nd[ci, ii, jj, :]                           # [NC, IT, PASSES]
            cand = loads + d_j[:, :, None, :]
            score = (cand.max(axis=3) * 4096 + cand.sum(axis=3)
                     + (sizes >= 16) * (1 << 50))
            g = score.argmin(axis=2)                          # [NC, IT]
            gsel[:, :, j] = g
            loads[ci, ii, g] += d_j
            sizes[ci, ii, g] += 1
        return gsel, loads.max(axis=2).sum(axis=2)            # [NC, IT]

    # multi-restart: different placement orders, keep best per (core, iter)
    rng = np.random.default_rng(12345)
    base = np.argsort(-nd.max(axis=3), axis=2, kind='stable')
    orders = [base, np.argsort(-nd.sum(axis=3), axis=2, kind='stable')]
    for _ in range(10):
        perm = rng.permuted(np.broadcast_to(
            np.arange(P), (NC, ITERS, P)).copy(), axis=2)
        orders.append(perm)
    best_score = None
    best_g = None
    best_ord = None
    for ordj in orders:
        gsel, sc = greedy(ordj)
        if best_score is None:
            best_score, best_g, best_ord = sc, gsel, ordj.copy()
        else:
            upd = sc < best_score
            best_score = np.where(upd, sc, best_score)
            best_g[upd] = gsel[upd]
            best_ord[upd] = ordj[upd]

    # rebuild placement from the chosen restart per (core, iter)
    nodes_gic = np.empty((NC, ITERS, P), np.int64)
    sizes = np.zeros((NC, ITERS, 8), np.int64)
    for j in range(P):
        jj = best_ord[:, :, j]
        g = best_g[:, :, j].astype(np.int64)
        slot = sizes[ci, ii, g]
        nodes_gic[ci, ii, 16 * g + slot] = nodes_rank[ci, ii, jj]
        sizes[ci, ii, g] += 1

    # inverse: real node -> (iteration, channel)
    it_of = np.empty(N + 1, np.int32)
    ch_of = np.empty(N + 1, np.int32)
    for c in range(NC):
        flat = nodes_gic[c].reshape(-1)
        it_of[flat] = np.arange(ITERS * P) // P
        ch_of[flat] = np.arange(ITERS * P) % P

    W = posx[nodes_gic].transpose(0, 2, 1).copy()             # [NC, 128, 98]
    cntf = degx[nodes_gic].transpose(0, 2, 1).astype(np.float32)

    # per-edge placement
    e_core = dst32 // NPC
    e_it = it_of[dst32]
    e_ch = ch_of[dst32]
    e_grp = e_ch // 16
    e_r = e_ch % 16
    e_pass = src32 // CPC
    e_sidx = (src32 - e_pass * CPC).astype(np.int16)

    # group-stream length per (core, pass, group, iter) then shared width
    key = (((e_core.astype(np.int64) * PASSES + e_pass) * 8 + e_grp)
           * ITERS + e_it)
    glen = np.bincount(key, minlength=NC * PASSES * 8 * ITERS)
    glen = glen.reshape(NC, PASSES, 8, ITERS)
    # shared L per (pass, iter): max over cores and groups (no per-window
    # rounding -- masks/sidx are absolute-slot addressed; only each chunk's
    # total is padded to a multiple of 16 for the wrapped idx layout)
    L_pi = np.maximum(glen.max(axis=(0, 2)), 1).astype(np.int64)

    # chunking: pack iterations so the padded sum <= IDX_CAP
    chunks = []                                               # per pass: (it0, it1)
    for p in range(PASSES):
        ch_list = []
        it0 = 0
        while it0 < ITERS:
            tot = 0
            it1 = it0
            while it1 < ITERS and ((tot + int(L_pi[p, it1]) + 15) // 16 * 16
                                   <= IDX_CAP):
                tot += int(L_pi[p, it1])
                it1 += 1
            ch_list.append((it0, it1))
            it0 = it1
        chunks.append(ch_list)

    # column offsets per (pass, iter); chunks padded to x16
    colof = np.zeros((PASSES, ITERS), np.int64)
    chunk_cols = {}                                           # (p, it0) -> padded cols
    off = 0
    for p in range(PASSES):
        for (it0, it1) in chunks[p]:
            c0 = off
            for it in range(it0, it1):
                colof[p, it] = off
                off += int(L_pi[p, it])
            off = (off + 15) // 16 * 16
            chunk_cols[(p, it0)] = off - c0
    total_cols = off                                          # slots per group

    # edge slot position: order edges by (core, pass, grp, it, r) and number
    order = np.lexsort((e_r, e_it, e_grp, e_pass, e_core))
    ks = key[order]
    run_start = np.r_[True, ks[1:] != ks[:-1]]
    pos_in_grp = np.arange(len(order)) - np.maximum.accumulate(
        np.where(run_start, np.arange(len(order)), 0))
    # slot of each (sorted) edge within its (pass, it) stream window
    oc = e_core[order]
    op_ = e_pass[order]
    og = e_grp[order]
    oi = e_it[order]
    orr = e_r[order]
    slot = colof[op_, oi] + pos_in_grp                        # [E] global col

    # sidx [NC, 128, total_cols/16] int16, wrapped per group;
    # mask [NC, 128, total_cols] bf16
    sidx = np.zeros((NC, P, total_cols // 16), np.int16)
    mask = np.zeros((NC, P, total_cols), BF)
    # wrapped position: stream slot t -> (partition 16g + t%16, col t//16)
    sidx[oc, 16 * og + slot % 16, slot // 16] = e_sidx[order]
    mask[oc, 16 * og + orr, slot] = 1.0

    # pass tables [NC, PASSES, 128, CPC] f32 (pos chunk replicated; pad slots
    # are masked so table[0] garbage is harmless)
    tbl = np.empty((NC, PASSES, P, CPC), np.float32)
    for p in range(PASSES):
        tbl[:, p, :, :] = pos[p * CPC:(p + 1) * CPC][None, None, :]

    in_maps = []
    for c in range(NC):
        in_maps.append({
            "tbl": tbl[c].reshape(PASSES * P, CPC),
            "sidx": sidx[c],
            "mask": mask[c],
            "wtab": W[c],
            "cntf": cntf[c],
        })
    meta = dict(chunks=chunks, L_pi=L_pi, colof=colof, chunk_cols=chunk_cols,
                total_cols=int(total_cols), nodes_gic=nodes_gic)
    return in_maps, meta


def _build_program(meta):
    chunks, L_pi, total_cols = meta["chunks"], meta["L_pi"], meta["total_cols"]
    chunk_cols = meta["chunk_cols"]
    nc = bacc.Bacc()
    tbl = nc.declare_dram_parameter("tbl", [PASSES * P, CPC],
                                    mybir.dt.float32, isOutput=False)
    sidx = nc.declare_dram_parameter("sidx", [P, total_cols // 16],
                                     mybir.dt.int16, isOutput=False)
    mask = nc.declare_dram_parameter("mask", [P, total_cols],
                                     mybir.dt.bfloat16, isOutput=False)
    wtab = nc.declare_dram_parameter("wtab", [P, ITERS], mybir.dt.float32,
                                     isOutput=False)
    cntf = nc.declare_dram_parameter("cntf", [P, ITERS], mybir.dt.float32,
                                     isOutput=False)
    out = nc.declare_dram_parameter("out", [P, 2 * ITERS], mybir.dt.float32,
                                    isOutput=True)
    outv = out[:].rearrange("p (b a) -> p b a", b=2)

    with TileContext(nc) as tc:
        with (
            tc.tile_pool(name="persist", bufs=1) as pers,
            tc.tile_pool(name="tblp", bufs=2) as tblp,
            tc.tile_pool(name="idxp", bufs=4) as idxp,
            tc.tile_pool(name="maskp", bufs=3) as maskp,
            tc.tile_pool(name="vp", bufs=3) as vp,
            tc.tile_pool(name="tbp", bufs=2) as tbp,
            tc.tile_pool(name="sp", bufs=2) as sp,
        ):
            t_w = pers.tile([P, ITERS], mybir.dt.float32, tag="t_w")
            t_cnt = pers.tile([P, ITERS], mybir.dt.float32, tag="t_cnt")
            t_s = pers.tile([P, ITERS], mybir.dt.float32, tag="t_s")
            # pass-0 table first: it gates the first gather
            t_tbl0 = tblp.tile([P, CPC], mybir.dt.float32, tag="tbl")
            nc.sync.dma_start(out=t_tbl0[:], in_=tbl[0:P])
            nc.sync.dma_start(out=t_w[:], in_=wtab[:])
            nc.sync.dma_start(out=t_cnt[:], in_=cntf[:])
            nc.vector.memset(t_s[:], 0.0)

            off = 0
            for p in range(PASSES):
                if p == 0:
                    t_tbl = t_tbl0
                else:
                    t_tbl = tblp.tile([P, CPC], mybir.dt.float32, tag="tbl")
                    nc.sync.dma_start(out=t_tbl[:],
                                      in_=tbl[p * P:(p + 1) * P])
                s_cols = sp.tile([P, ITERS], mybir.dt.float32, tag="scols")
                nc.vector.memset(s_cols[:], 0.0)
                for (it0, it1) in chunks[p]:
                    Ls = [int(L_pi[p, it]) for it in range(it0, it1)]
                    cols = chunk_cols[(p, it0)]
                    si = idxp.tile([P, cols // 16], mybir.dt.int16, tag="si")
                    nc.sync.dma_start(out=si[:],
                                      in_=sidx[:, off // 16:(off + cols) // 16])
                    mk = maskp.tile([P, cols], mybir.dt.bfloat16, tag="mk")
                    nc.sync.dma_start(out=mk[:], in_=mask[:, off:off + cols])
                    v = vp.tile([P, cols], mybir.dt.float32, tag="v")
                    nc.gpsimd.ap_gather(out_ap=v[:], in_ap=t_tbl[:],
                                        idxs_ap=si[:], channels=P,
                                        num_elems=CPC, d=1, num_idxs=cols)
                    tb = tbp.tile([P, cols], mybir.dt.bfloat16, tag="tb")
                    co = 0
                    for k, it in enumerate(range(it0, it1)):
                        L = Ls[k]
                        nc.vector.tensor_scalar(
                            out=tb[:, co:co + L], in0=v[:, co:co + L],
                            scalar1=t_w[:, it:it + 1], scalar2=None,
                            op0=mybir.AluOpType.subtract)
                        nc.vector.tensor_tensor(
                            out=tb[:, co:co + L], in0=tb[:, co:co + L],
                            in1=mk[:, co:co + L], op=mybir.AluOpType.mult)
                        nc.vector.tensor_reduce(
                            out=s_cols[:, it:it + 1], in_=tb[:, co:co + L],
                            axis=mybir.AxisListType.X, op=mybir.AluOpType.add,
                            apply_absolute_value=True)
                        co += L
                    off += cols
                nc.vector.tensor_tensor(out=t_s[:], in0=t_s[:], in1=s_cols[:],
                                        op=mybir.AluOpType.add)

            nc.vector.tensor_scalar_max(out=t_cnt[:], in0=t_cnt[:],
                                        scalar1=1.0)
            nc.vector.reciprocal(out=t_cnt[:], in_=t_cnt[:])
            nc.vector.tensor_tensor(out=t_s[:], in0=t_s[:], in1=t_cnt[:],
                                    op=mybir.AluOpType.mult)
            nc.sync.dma_start(out=outv[:, 0], in_=t_w[:])
            nc.sync.dma_start(out=outv[:, 1], in_=t_s[:])

    nc.compile()
    _split_sync_waits(nc)
    return nc


def kernel(h, src, dst):
    h = np.asarray(h)
    src = np.asarray(src)
    dst = np.asarray(dst)
    in_maps, meta = _host_prep(h, src, dst)
    nc = _build_program(meta)
    res = run_bass_kernel_spmd(nc, in_maps, list(range(NC)))
    nodes_gic = meta["nodes_gic"]
    final = np.empty((N_NODES, 2), np.float32)
    for c in range(NC):
        r = res.results[c]["out"].reshape(P, 2, ITERS)
        flat_nodes = nodes_gic[c].reshape(-1)          # (it, ch) order
        vals = r.transpose(2, 0, 1).reshape(-1, 2)
        valid = flat_nodes < N_NODES                   # drop sentinel dummies
        final[flat_nodes[valid]] = vals[valid]
    return final
